# revision 1
# baseline (speedup 1.0000x reference)
"""GRU Trainium kernel builder + host-side data packing.

Per-core problem: B=32, T steps, H=512, 2 layers, gates [r,z,n].
Layout decisions (see design notes):
  - matmul option A: out[b, h] in psum, lhsT = h^T (bf16), rhs = W (bf16).
  - col-tiling: 4 strips x 32 partitions; units:
      bankA strips: s0=r0, s1=r1, s2=z0, s3=z1
      bankB strips: s0=ni0, s1=ni1, s2=nh0, s3=nh1
  - gate math packed over both layers: [64, 512] ops; SBUF intermediates:
      rz_sb [128,512] (sigma out: parts 0:64 = r(l0,l1), 64:128 = z(l0,l1))
      tmp/u/n at parts 0:64; v at parts 64:128; w at 0:64; h' at 0:64.
  - h state: h_sb [64,512] fp32 (parts 0:32 = h0, 32:64 = h1), ping-pong.
  - h^T state: per layer ring of [128, 128] bf16 tiles (4 k-chunks x 32 batch).
  - L1 runs SKEW steps behind L0.
"""
import numpy as np
import ml_dtypes
import concourse.bass as bass
from concourse import bacc
import concourse.tile as tile
import concourse.mybir as mybir

F32 = mybir.dt.float32
BF16 = mybir.dt.bfloat16
AF = mybir.ActivationFunctionType

H = 512
BL = 32          # batch per core
NK = 4           # k-chunks of 128
RING = 4         # hT ring depth


def build_gru(T=512, skew=2, n_cores=8, pool_ops=True):
    """Returns compiled Bacc module. DRAM tensor names/shapes:
      inputs: xw [2, T*32] bf16, wh0/wh1/wi1 [128, 6144] bf16,
              wi0a [2, 1536] bf16, biasv [1, 2562] bf16, wfc [128, 8] bf16
      output: out [32, 2] f32
    """
    nc = bacc.Bacc("TRN2", target_bir_lowering=False, debug=False,
                   num_devices=n_cores)
    xw_d = nc.dram_tensor("xw", (2, T * BL), BF16, kind="ExternalInput").ap()
    wh0_d = nc.dram_tensor("wh0", (128, 12 * H), BF16, kind="ExternalInput").ap()
    wh1_d = nc.dram_tensor("wh1", (128, 12 * H), BF16, kind="ExternalInput").ap()
    wi1_d = nc.dram_tensor("wi1", (128, 12 * H), BF16, kind="ExternalInput").ap()
    wi0a_d = nc.dram_tensor("wi0a", (2, 3 * H), BF16, kind="ExternalInput").ap()
    bias_d = nc.dram_tensor("biasv", (1, 5 * H + 2), BF16, kind="ExternalInput").ap()
    wfc_d = nc.dram_tensor("wfc", (128, 8), BF16, kind="ExternalInput").ap()
    out_d = nc.dram_tensor("out", (BL, 2), F32, kind="ExternalOutput").ap()

    with tile.TileContext(nc) as tc:
        import contextlib
        with contextlib.ExitStack() as ctx:
            const = ctx.enter_context(tc.tile_pool(name="const", bufs=1))
            state = ctx.enter_context(tc.tile_pool(name="state", bufs=1))
            scratch = ctx.enter_context(tc.tile_pool(name="scratch", bufs=2))
            pspool = ctx.enter_context(tc.tile_pool(name="ps", bufs=2, space="PSUM"))
            psfc = ctx.enter_context(tc.tile_pool(name="psfc", bufs=1, space="PSUM"))

            # ---- persistent tiles ----
            xw = const.tile([2, T * BL], BF16)
            wh0 = const.tile([128, 12 * H], BF16)
            wh1 = const.tile([128, 12 * H], BF16)
            wi1 = const.tile([128, 12 * H], BF16)
            wi0a = const.tile([2, 3 * H], BF16)
            biasv = const.tile([1, 5 * H + 2], BF16)
            wfc = const.tile([128, 8], BF16)
            for t_, d_ in [(xw, xw_d), (wh0, wh0_d), (wh1, wh1_d), (wi1, wi1_d),
                           (wi0a, wi0a_d), (biasv, bias_d), (wfc, wfc_d)]:
                nc.sync.dma_start(out=t_[:], in_=d_)

            id4 = const.tile([128, 32], F32)
            from concourse.masks import make_identity
            for j_ in range(4):
                make_identity(nc, id4[32 * j_:32 * (j_ + 1), :])

            # h state ping-pong [64, 512] fp32 and hT rings [128,128] bf16
            h_sb = [state.tile([64, H], F32, name=f"h{i}", tag=f"h{i}") for i in range(2)]
            h0T = [state.tile([128, NK * BL], BF16, name=f"h0T{i}", tag=f"h0T{i}") for i in range(RING)]
            h1T = [state.tile([128, NK * BL], BF16, name=f"h1T{i}", tag=f"h1T{i}") for i in range(RING)]
            for t_ in h_sb + h0T + h1T:
                nc.vector.memset(t_[:], 0.0)

            # weight slices helper: w tile, gate g, kchunk c -> [128, 512] rhs
            def wsl(w, g, c):
                return w[:, (3 * c + g) * H:(3 * c + g + 1) * H]

            ones_t = const.tile([1, BL], BF16)
            nc.vector.memset(ones_t[:], 1.0)
            ones_lhs = ones_t[0:1, 0:BL]  # [1, 32] of ones at partition 0

            def bias_rhs(idx):  # idx in {0..4}: bh0n, br1, bz1, bi1n, bh1n
                return biasv[0:1, idx * H:(idx + 1) * H]

            # ---------------- superstep loop ----------------
            n_super = T + skew
            for s in range(n_super):
                l0 = s < T
                l1 = s >= skew
                t0 = s           # L0 timestep
                t1 = s - skew    # L1 timestep
                par = s % 2

                bankA = pspool.tile([128, H], F32, tag="bankA")
                bankB = pspool.tile([128, H], F32, tag="bankB")
                trps = pspool.tile([128, 2 * NK * BL], F32, tag="trps")

                # hT operands (written at end of superstep st for timestep st)
                h0T_l0 = h0T[(t0 - 1) % RING] if t0 >= 1 else None   # h0(t0-1)
                h0T_l1 = h0T[t1 % RING] if l1 else None              # h0(t1)
                h1T_l1 = h1T[(t1 - 1) % RING] if t1 >= 1 else None   # h1(t1-1)

                # ---- build unit MM lists: (psum_slice, strip, [(lhsT, rhs)...]) ----
                # order within unit: independent (gi/bias) first, hid last
                units = []  # list of (out_ap, tile_col, mms)
                if l0:
                    # r0 @ bankA s0 ; z0 @ bankA s2 ; ni0 @ bankB s0 ; nh0 @ bankB s2
                    xt = xw[0:2, BL * t0: BL * (t0 + 1)]  # K=2 lhsT (x_t, ones)
                    r0 = [(xt, wi0a[0:2, 0:H])]
                    z0 = [(xt, wi0a[0:2, H:2 * H])]
                    ni0 = [(xt, wi0a[0:2, 2 * H:3 * H])]
                    nh0 = [(ones_lhs, bias_rhs(0))]
                    if t0 >= 1:
                        for c in range(NK):
                            lh = h0T_l0[:, BL * c: BL * (c + 1)]
                            r0.append((lh, wsl(wh0, 0, c)))
                            z0.append((lh, wsl(wh0, 1, c)))
                            nh0.append((lh, wsl(wh0, 2, c)))
                    units += [(bankA[0:32, :], 0, r0), (bankA[64:96, :], 64, z0),
                              (bankB[0:32, :], 0, ni0), (bankB[64:96, :], 64, nh0)]
                if l1:
                    r1 = [(ones_lhs, bias_rhs(1))]
                    z1 = [(ones_lhs, bias_rhs(2))]
                    ni1 = [(ones_lhs, bias_rhs(3))]
                    nh1 = [(ones_lhs, bias_rhs(4))]
                    for c in range(NK):
                        lh = h0T_l1[:, BL * c: BL * (c + 1)]
                        r1.append((lh, wsl(wi1, 0, c)))
                        z1.append((lh, wsl(wi1, 1, c)))
                        ni1.append((lh, wsl(wi1, 2, c)))
                    if t1 >= 1:
                        for c in range(NK):
                            lh = h1T_l1[:, BL * c: BL * (c + 1)]
                            r1.append((lh, wsl(wh1, 0, c)))
                            z1.append((lh, wsl(wh1, 1, c)))
                            nh1.append((lh, wsl(wh1, 2, c)))
                    units += [(bankA[32:64, :], 32, r1), (bankA[96:128, :], 96, z1),
                              (bankB[32:64, :], 32, ni1), (bankB[96:128, :], 96, nh1)]

                # ---- emit MMs round-robin across strips ----
                by_strip = {}
                for out_ap, col, mms in units:
                    by_strip.setdefault(col, []).append((out_ap, mms, [False]))
                # flatten: per strip, a queue of (out_ap, mm, is_first, is_last)
                queues = {}
                for col, us in by_strip.items():
                    qi, qd = [], []
                    for out_ap, mms, _ in us:
                        n_indep = len(mms) - (NK if (mms and mms[-1][1].tensor.name.startswith(("wh0", "wh1"))) else 0)
                        for i, mm in enumerate(mms):
                            ent = (out_ap, mm, i == 0, i == len(mms) - 1)
                            (qi if i < n_indep else qd).append(ent)
                    queues[col] = qi + qd
                maxlen = max(len(q) for q in queues.values())
                for i in range(maxlen):
                    for col in sorted(queues):
                        q = queues[col]
                        if i < len(q):
                            out_ap, (lh, rh), first, last = q[i]
                            nc.tensor.matmul(out_ap, lhsT=lh, rhs=rh,
                                             start=first, stop=last,
                                             tile_position=(0, col),
                                             skip_group_check=True)

                # ---- gate math ----
                # active partition windows
                if l0 and l1:
                    lo, hi = 0, 64
                elif l0:
                    lo, hi = 0, 32
                else:
                    lo, hi = 32, 64
                n_act = hi - lo

                rz = scratch.tile([128, H], F32, tag="rz")
                tmp = scratch.tile([64, H], F32, tag="tmp")
                u = scratch.tile([64, H], F32, tag="u")
                nn_ = scratch.tile([64, H], F32, tag="nn")
                vw = scratch.tile([128, H], F32, tag="vw")
                hnew = h_sb[par]
                hold = h_sb[1 - par]

                # sigma over r and z regions of bankA (restrict to active windows)
                nc.scalar.activation(rz[lo:hi, :], bankA[lo:hi, :], AF.Sigmoid)
                nc.scalar.activation(rz[64 + lo:64 + hi, :], bankA[64 + lo:64 + hi, :], AF.Sigmoid)
                # tmp = r * nh   (SBUF x PSUM, windows may differ)
                nc.vector.tensor_mul(out=tmp[lo:hi, :], in0=rz[lo:hi, :], in1=bankB[64 + lo:64 + hi, :])
                # u = tmp + ni
                nc.vector.tensor_add(out=u[lo:hi, :], in0=tmp[lo:hi, :], in1=bankB[lo:hi, :])
                # n = tanh(u)
                nc.scalar.activation(nn_[lo:hi, :], u[lo:hi, :], AF.Tanh)
                # v = h_old - n   -> parts 64+
                eng_a = nc.gpsimd if pool_ops else nc.vector
                eng_b = nc.gpsimd if pool_ops else nc.vector
                eng_a.tensor_sub(out=vw[64 + lo:64 + hi, :], in0=hold[lo:hi, :], in1=nn_[lo:hi, :])
                # w = z * v -> parts 0:64 of vw
                eng_b.tensor_mul(out=vw[lo:hi, :], in0=rz[64 + lo:64 + hi, :], in1=vw[64 + lo:64 + hi, :])
                # h' = n + w
                nc.vector.tensor_add(out=hnew[lo:hi, :], in0=nn_[lo:hi, :], in1=vw[lo:hi, :])

                # ---- transposes: h'(layer) [32,512] -> hT [128, 128] bf16 ----
                ident = None
                for (active, base, ring, tstep) in [
                    (l0, 0, h0T, t0), (l1, 32, h1T, t1)]:
                    if not active:
                        continue
                    dst = ring[tstep % RING]
                    off = 0 if base == 0 else NK * BL
                    for c in range(NK):
                        nc.tensor.transpose(
                            trps[:, off + BL * c: off + BL * (c + 1)],
                            hnew[base:base + 32, 128 * c:128 * (c + 1)],
                            id4[base:base + 32, :],
                            tile_position=(base, 0),
                        )
                    nc.vector.tensor_copy(out=dst[:], in_=trps[:, off:off + NK * BL])

            # ---- FC ----
            ps_fc = psfc.tile([BL, 2], F32)
            hT_last = h1T[(T - 1) % RING]
            for c in range(NK):
                nc.tensor.matmul(ps_fc[:, :], lhsT=hT_last[:, BL * c:BL * (c + 1)],
                                 rhs=wfc[:, 2 * c:2 * (c + 1)],
                                 start=(c == 0), stop=False, skip_group_check=True)
            nc.tensor.matmul(ps_fc[:, :], lhsT=ones_lhs,
                             rhs=biasv[0:1, 5 * H:5 * H + 2],
                             start=False, stop=True, skip_group_check=True)
            out_sb = const.tile([BL, 2], F32)
            nc.vector.tensor_copy(out=out_sb[:], in_=ps_fc[:, :])
            nc.sync.dma_start(out=out_d, in_=out_sb[:])

    nc.compile()
    return nc


# ---------------- host-side packing ----------------

def pack_inputs(x, Wi0, bi0, Wi_rest, bi_rest, Wh, bh, fc_w, fc_b, n_cores=8):
    """Full inputs -> list of per-core in_maps."""
    B, T = x.shape
    bl = B // n_cores
    assert bl == BL

    def w_pack(W3):  # [3, H, H] -> [128, 12*H] with [p, (3c+g)*H + n] = W3[g, n, 128c+p]
        a = W3.transpose(2, 0, 1)            # [i, g, n]
        a = a.reshape(NK, 128, 3, H)         # [c, p, g, n]
        a = a.transpose(1, 0, 2, 3)          # [p, c, g, n]
        return np.ascontiguousarray(a).reshape(128, 12 * H).astype(ml_dtypes.bfloat16)

    wh0 = w_pack(Wh[0]); wh1 = w_pack(Wh[1]); wi1 = w_pack(Wi_rest[0])

    wi0a = np.zeros((2, 3 * H), np.float32)
    for g in range(3):
        wi0a[0, g * H:(g + 1) * H] = Wi0[g, :, 0]
    wi0a[1, 0:H] = bi0[0] + bh[0, 0]
    wi0a[1, H:2 * H] = bi0[1] + bh[0, 1]
    wi0a[1, 2 * H:3 * H] = bi0[2]
    wi0a = wi0a.astype(ml_dtypes.bfloat16)

    biasv = np.zeros((1, 5 * H + 2), np.float32)
    biasv[0, 0:H] = bh[0, 2]
    biasv[0, H:2 * H] = bi_rest[0, 0] + bh[1, 0]
    biasv[0, 2 * H:3 * H] = bi_rest[0, 1] + bh[1, 1]
    biasv[0, 3 * H:4 * H] = bi_rest[0, 2]
    biasv[0, 4 * H:5 * H] = bh[1, 2]
    biasv[0, 5 * H:] = fc_b
    biasv = biasv.astype(ml_dtypes.bfloat16)

    wfc = fc_w.T.reshape(NK, 128, 2).transpose(1, 0, 2)
    wfc = np.ascontiguousarray(wfc).reshape(128, 8).astype(ml_dtypes.bfloat16)

    in_maps = []
    for c in range(n_cores):
        xc = x[c * bl:(c + 1) * bl, :]       # [32, T]
        xw = np.empty((2, T * bl), np.float32)
        xw[0] = xc.T.reshape(-1)             # [t*32 + b]
        xw[1] = 1.0
        in_maps.append({
            "xw": xw.astype(ml_dtypes.bfloat16),
            "wh0": wh0, "wh1": wh1, "wi1": wi1,
            "wi0a": wi0a, "biasv": biasv, "wfc": wfc,
        })
    return in_maps


def unpack_outputs(results):
    return np.concatenate([r["out"] for r in results], axis=0)


# ---------------- public entry point ----------------
_CACHED = {}

def _get_nc(T):
    if T not in _CACHED:
        _CACHED[T] = build_gru(T=T)
    return _CACHED[T]


def kernel(x, Wi0, bi0, Wi_rest, bi_rest, Wh, bh, fc_w, fc_b):
    """Full-input 2-layer GRU (B=256, H=512) on 8 NeuronCores.

    Sharding: data-parallel over batch (32 per core), weights replicated.
    Inside: bf16 matmuls (col-tiled option-A layout), fp32 psum/state,
    per-step gate math packed across both layers, PE transposes for h^T.
    """
    from concourse.bass_utils import run_bass_kernel_spmd
    x = np.asarray(x); Wi0 = np.asarray(Wi0); bi0 = np.asarray(bi0)
    Wi_rest = np.asarray(Wi_rest); bi_rest = np.asarray(bi_rest)
    Wh = np.asarray(Wh); bh = np.asarray(bh)
    fc_w = np.asarray(fc_w); fc_b = np.asarray(fc_b)
    T = x.shape[1]
    nc = _get_nc(T)
    in_maps = pack_inputs(x, Wi0, bi0, Wi_rest, bi_rest, Wh, bh, fc_w, fc_b)
    res = run_bass_kernel_spmd(nc, in_maps, core_ids=list(range(8)))
    return unpack_outputs(res.results).astype(np.float32)

